# revision 5
# baseline (speedup 1.0000x reference)
"""Trainium2 Bass kernel for nn_MultiHeadAttention (B=2, S=2048, D=1024, H=16).

Sharding (8 cores): batch (2-way) x head-group (4-way).
Core c: batch b=c//4, head-group hg=c%4 (4 heads = 256 of d_model).
Megatron style: Wq/Wk/Wv column-parallel, Wo row-parallel; the 4 partial
outputs per batch are summed on the host (plus b_o).

Per-core device pipeline (all matmuls f32r = TF32-like, 1 cyc/row):
  phase 1: project qhT/khT [do,t] and vh [t,do] from host-pre-transposed
           qT/kT/vT chunks; biases fused (ACT bias operand / K=1 matmul).
  phase 2: per 512-query chunk x head-pair: scoresT[kj,qi] via K=64
           matmuls packed 2-heads-per-PE-pass (tile_position row strips),
           exp on ACT (scale 1/8 folded, bf16 out), keep-mask multiply on
           DVE (bf16 2x mode), PV accumulation with an appended ones
           column so row-sums ride along; normalize after a PE transpose
           (per-partition reciprocal), transpose back for o-proj layout.
  phase 3: o-proj into natural [t, d_model] layout, DMA out.
"""
import os

if "JAX_PLATFORMS" in os.environ and "axon" not in os.environ["JAX_PLATFORMS"]:
    del os.environ["JAX_PLATFORMS"]

import numpy as np
import ml_dtypes

B, S, D = 2, 2048, 1024
H, DK = 16, 64
NCORES = 8
HGROUPS = 4               # head-groups (cores per batch)
DLOC = D // HGROUPS       # 256 dims per core
NHL = DLOC // DK          # 4 local heads
NKT = D // 128            # 8 k-tiles over d_model
TCH = 512                 # token chunk
NCH = S // TCH            # 4 chunks
NT = S // 128             # 16 token tiles
NKJ = S // 128            # 16 key tiles
SCALE = 1.0 / 8.0         # 1/sqrt(DK)

_CACHE = {}


def _build(reps=1):
    """Trace + compile the per-core Bass kernel (cached).

    reps>1 wraps the whole body in a tc.For_i hardware loop (timing use).
    """
    key = ("nc", reps)
    if key in _CACHE:
        return _CACHE[key]
    import concourse.bacc as bacc
    import concourse.mybir as mybir
    from concourse.tile import TileContext

    f32r = mybir.dt.float32r
    f32 = mybir.dt.float32
    f16 = mybir.dt.float16
    AF = mybir.ActivationFunctionType

    nc = bacc.Bacc("TRN2", target_bir_lowering=False)

    qT_d = nc.dram_tensor("qT", [D, S], f32r, kind="ExternalInput")
    kT_d = nc.dram_tensor("kT", [D, S], f32r, kind="ExternalInput")
    vT_d = nc.dram_tensor("vT", [D, S], f32r, kind="ExternalInput")
    mk_d = nc.dram_tensor("maskT", [S, S], f16, kind="ExternalInput")
    wq_d = nc.dram_tensor("wq", [D, DLOC], f32r, kind="ExternalInput")
    wk_d = nc.dram_tensor("wk", [D, DLOC], f32r, kind="ExternalInput")
    wv_d = nc.dram_tensor("wv", [D, DLOC], f32r, kind="ExternalInput")
    wo_d = nc.dram_tensor("wo", [DLOC, D], f32r, kind="ExternalInput")
    bq_d = nc.dram_tensor("bq", [128, 2], f32r, kind="ExternalInput")
    bk_d = nc.dram_tensor("bk", [128, 2], f32r, kind="ExternalInput")
    bv_d = nc.dram_tensor("bv", [1, DLOC], f32r, kind="ExternalInput")
    id_d = nc.dram_tensor("ident", [128, 128], f32r, kind="ExternalInput")
    ones1_d = nc.dram_tensor("ones1", [1, S], f32r, kind="ExternalInput")
    ones2_d = nc.dram_tensor("ones2", [128, NT, NHL, 2], f16,
                             kind="ExternalInput")
    out_d = nc.dram_tensor("out", [S, D], f32, kind="ExternalOutput")

    qT_r = qT_d.rearrange("(kt p) t -> p kt t", p=128)
    kT_r = kT_d.rearrange("(kt p) t -> p kt t", p=128)
    vT_r = vT_d.rearrange("(kt p) t -> p kt t", p=128)
    mk_r = mk_d.rearrange("(j p) q -> p j q", p=128)

    with TileContext(nc) as tc:
        with (
            tc.tile_pool(name="big", bufs=1) as big,
            tc.tile_pool(name="xin", bufs=2) as xin,
            tc.tile_pool(name="mp", bufs=2) as mp,
            tc.tile_pool(name="ep", bufs=3) as ep,
            tc.tile_pool(name="sp", bufs=3) as sp,
            tc.tile_pool(name="ps", bufs=1, space="PSUM") as ps,
        ):
          import contextlib
          loop_cm = tc.For_i(0, reps, 1) if reps > 1 else contextlib.nullcontext()
          with loop_cm:
            # ---- constants / weights ----
            wq_sb = big.tile([128, NKT, DLOC], f32r)
            wk_sb = big.tile([128, NKT, DLOC], f32r)
            wv_sb = big.tile([128, NKT, DLOC], f32r)
            wo_sb = big.tile([128, DLOC // 128, D], f32r)
            nc.sync.dma_start(out=wq_sb, in_=wq_d.rearrange("(kt p) o -> p kt o", p=128))
            nc.sync.dma_start(out=wk_sb, in_=wk_d.rearrange("(kt p) o -> p kt o", p=128))
            nc.sync.dma_start(out=wv_sb, in_=wv_d.rearrange("(kt p) o -> p kt o", p=128))
            nc.sync.dma_start(out=wo_sb, in_=wo_d.rearrange("(kk p) o -> p kk o", p=128))
            bq_sb = big.tile([128, 2], f32r)
            bk_sb = big.tile([128, 2], f32r)
            bv_sb = big.tile([1, DLOC], f32r)
            id_sb = big.tile([128, 128], f32r)
            ones1_sb = big.tile([1, S], f32r)
            nc.sync.dma_start(out=bq_sb, in_=bq_d[:, :])
            nc.sync.dma_start(out=bk_sb, in_=bk_d[:, :])
            nc.sync.dma_start(out=bv_sb, in_=bv_d[:, :])
            nc.sync.dma_start(out=id_sb, in_=id_d[:, :])
            nc.sync.dma_start(out=ones1_sb, in_=ones1_d[:, :])

            # ---- persistent activations ----
            qhT_sb = big.tile([128, 2, S], f32r)     # [p, m, t]
            khT_sb = big.tile([128, 2, S], f32r)
            vh1_sb = big.tile([128, NT, NHL, DK + 2], f16)
            aoT_sb = big.tile([128, 2, S], f32r)     # normalized attnout^T
            nc.sync.dma_start(out=vh1_sb[:, :, :, DK:DK + 2], in_=ones2_d[:, :, :, :])

            # one 4-bank psum slot shared by qk-proj (quadrants) and scores
            s4 = ps.tile([128, 2, 2, TCH], f32, tag="s4", name="s4", bufs=1)

            # ---- phase 1: projections ----
            quad = 0
            for xname, xr, w_sb, b_sb, hT_sb in (
                ("k", kT_r, wk_sb, bk_sb, khT_sb),
                ("q", qT_r, wq_sb, bq_sb, qhT_sb),
            ):
                for tch in range(NCH):
                    xt = xin.tile([128, NKT, TCH], f32r, tag="xt",
                                  name=f"xt_{xname}{tch}")
                    nc.sync.dma_start(
                        out=xt, in_=xr[:, :, tch * TCH:(tch + 1) * TCH])
                    for m in range(2):
                        acc = s4[:, quad % 2, quad // 2 % 2, :]
                        for kt in range(NKT):
                            nc.tensor.matmul(
                                acc, w_sb[:, kt, m * 128:(m + 1) * 128],
                                xt[:, kt, :],
                                start=(kt == 0), stop=(kt == NKT - 1))
                        nc.vector.tensor_scalar_add(
                            out=hT_sb[:, m, tch * TCH:(tch + 1) * TCH],
                            in0=acc, scalar1=b_sb[:, m:m + 1].bitcast(f32))
                        quad += 1
            for tch in range(NCH):
                xt = xin.tile([128, NKT, TCH], f32r, tag="xt", name=f"xt_v{tch}")
                nc.sync.dma_start(
                    out=xt, in_=vT_r[:, :, tch * TCH:(tch + 1) * TCH])
                for mm in range(TCH // 128):
                    m16 = tch * (TCH // 128) + mm
                    pv = ps.tile([128, DLOC], f32, tag=f"pv{m16 % 2}",
                                 name=f"psv_{m16}", bufs=1)
                    for kt in range(NKT):
                        nc.tensor.matmul(
                            pv, xt[:, kt, mm * 128:(mm + 1) * 128],
                            wv_sb[:, kt, :],
                            start=(kt == 0), stop=False)
                    nc.tensor.matmul(
                        pv, ones1_sb[0:1, m16 * 128:(m16 + 1) * 128],
                        bv_sb[0:1, :], start=False, stop=True)
                    nc.vector.tensor_copy(
                        vh1_sb[:, m16, :, 0:DK],
                        pv.rearrange("p (h d) -> p h d", h=NHL))

            # ---- phase 2: attention ----
            for tcq in range(NCH):
                qsl = slice(tcq * TCH, (tcq + 1) * TCH)
                mk_sb = mp.tile([128, NKJ, TCH], f16, tag="mk",
                                name=f"mk_{tcq}")
                nc.sync.dma_start(out=mk_sb, in_=mk_r[:, :, qsl])
                for hp in range(2):
                    pvT = [
                        ps.tile([DK + 2, TCH], f32, tag=f"pv{hh}",
                                name=f"pvT_{tcq}_{hp}_{hh}", bufs=1)
                        for hh in range(2)
                    ]
                    s_sl = ps.tile([128, 2, 2, TCH], f32, tag="s4",
                                   name=f"s_{tcq}_{hp}", bufs=1)
                    for jg in range(NKJ // 2):
                        e_sb = ep.tile([128, 2, 2, TCH], f16, tag="e",
                                       name=f"e_{tcq}_{hp}_{jg}")
                        for jj in range(2):
                            j = jg * 2 + jj
                            for hh in range(2):
                                nc.tensor.matmul(
                                    s_sl[:, jj, hh, :],
                                    khT_sb[64 * hh:64 * (hh + 1), hp,
                                           j * 128:(j + 1) * 128],
                                    qhT_sb[64 * hh:64 * (hh + 1), hp, qsl],
                                    start=True, stop=True,
                                    tile_position=(64 * hh, 0))
                            nc.scalar.activation(
                                out=e_sb[:, jj, :, :], in_=s_sl[:, jj, :, :],
                                func=AF.Exp, scale=SCALE)
                        for hh in range(2):
                            nc.vector.tensor_mul(
                                e_sb[:, :, hh, :], e_sb[:, :, hh, :],
                                mk_sb[:, jg * 2:jg * 2 + 2, :])
                        for jj in range(2):
                            j = jg * 2 + jj
                            for hh in range(2):
                                nc.tensor.matmul(
                                    pvT[hh], vh1_sb[:, j, hp * 2 + hh, :],
                                    e_sb[:, jj, hh, :],
                                    start=(j == 0), stop=(j == NKJ - 1))
                    # finalize pair: normalize + transpose back
                    for hh in range(2):
                        pvT_sb = sp.tile([DK + 2, TCH], f32r, tag="pvs",
                                         name=f"pvs_{tcq}_{hp}_{hh}")
                        nc.vector.tensor_copy(pvT_sb, pvT[hh])
                        t_sl = ps.tile([128, TCH // 128, DK + 2], f32r,
                                       tag="t", name=f"t_{tcq}_{hp}_{hh}",
                                       bufs=2)
                        for i in range(TCH // 128):
                            nc.tensor.transpose(
                                t_sl[:, i, :],
                                pvT_sb[:, i * 128:(i + 1) * 128],
                                id_sb[0:DK + 2, 0:DK + 2])
                        rcp = sp.tile([128, TCH // 128], f32, tag="rcp",
                                      name=f"rcp_{tcq}_{hp}_{hh}")
                        nc.vector.reciprocal(
                            rcp, t_sl[:, :, DK].bitcast(f32))
                        ao = sp.tile([128, TCH // 128, DK], f32r, tag="ao",
                                     name=f"ao_{tcq}_{hp}_{hh}")
                        for i in range(TCH // 128):
                            nc.vector.tensor_scalar_mul(
                                ao[:, i, :], t_sl[:, i, 0:DK],
                                rcp[:, i:i + 1])
                        aoT_sl = ps.tile([DK, TCH // 128, 128], f32r,
                                         tag="t", name=f"aoT_{tcq}_{hp}_{hh}",
                                         bufs=2)
                        for i in range(TCH // 128):
                            nc.tensor.transpose(
                                aoT_sl[:, i, :], ao[:, i, :], id_sb)
                        nc.vector.tensor_copy(
                            aoT_sb[64 * hh:64 * (hh + 1), hp, qsl].rearrange(
                                "p (i q) -> p i q", i=TCH // 128),
                            aoT_sl)

            # ---- phase 3: o-proj ----
            for m16 in range(NT):
                o_sb = sp.tile([128, D], f32, tag="o", name=f"o_{m16}")
                for n in range(2):
                    po = ps.tile([128, 512], f32, tag="t",
                                 name=f"po_{m16}_{n}", bufs=2)
                    for kk in range(2):
                        nc.tensor.matmul(
                            po, aoT_sb[:, kk, m16 * 128:(m16 + 1) * 128],
                            wo_sb[:, kk, n * 512:(n + 1) * 512],
                            start=(kk == 0), stop=(kk == 1))
                    nc.vector.tensor_copy(o_sb[:, n * 512:(n + 1) * 512], po)
                nc.sync.dma_start(
                    out=out_d[m16 * 128:(m16 + 1) * 128, :], in_=o_sb)

    nc.compile()
    _CACHE[key] = nc
    return nc


def _in_maps(q, k, v, mask, w_q, b_q, w_k, b_k, w_v, b_v, w_o, b_o):
    q = np.asarray(q, dtype=np.float32)
    k = np.asarray(k, dtype=np.float32)
    v = np.asarray(v, dtype=np.float32)
    mask = np.asarray(mask)
    w_q = np.asarray(w_q, dtype=np.float32)
    w_k = np.asarray(w_k, dtype=np.float32)
    w_v = np.asarray(w_v, dtype=np.float32)
    w_o = np.asarray(w_o, dtype=np.float32)
    b_q = np.asarray(b_q, dtype=np.float32)
    b_k = np.asarray(b_k, dtype=np.float32)
    b_v = np.asarray(b_v, dtype=np.float32)

    hf = np.float16
    qT = [np.ascontiguousarray(q[b].T) for b in range(B)]
    kT = [np.ascontiguousarray(k[b].T) for b in range(B)]
    vT = [np.ascontiguousarray(v[b].T) for b in range(B)]
    mkT = [np.ascontiguousarray((~mask[b, 0]).T).astype(hf) for b in range(B)]
    ident = np.eye(128, dtype=np.float32)
    ones1 = np.ones((1, S), dtype=np.float32)
    ones2 = np.ones((128, NT, NHL, 2), dtype=hf)

    maps = []
    for c in range(NCORES):
        b, hg = c // HGROUPS, c % HGROUPS
        sl = slice(hg * DLOC, (hg + 1) * DLOC)
        maps.append({
            "qT": qT[b], "kT": kT[b], "vT": vT[b], "maskT": mkT[b],
            "wq": np.ascontiguousarray(w_q[:, sl]),
            "wk": np.ascontiguousarray(w_k[:, sl]),
            "wv": np.ascontiguousarray(w_v[:, sl]),
            "wo": np.ascontiguousarray(w_o[sl, :]),
            "bq": np.ascontiguousarray(b_q[sl].reshape(2, 128).T),
            "bk": np.ascontiguousarray(b_k[sl].reshape(2, 128).T),
            "bv": b_v[sl].reshape(1, DLOC).copy(),
            "ident": ident, "ones1": ones1, "ones2": ones2,
        })
    return maps


def kernel(q, k, v, mask, w_q, b_q, w_k, b_k, w_v, b_v, w_o, b_o):
    from concourse.bass_utils import run_bass_kernel_spmd

    nc = _build()
    maps = _in_maps(q, k, v, mask, w_q, b_q, w_k, b_k, w_v, b_v, w_o, b_o)
    res = run_bass_kernel_spmd(nc, maps, list(range(NCORES)))
    b_o = np.asarray(b_o, dtype=np.float32)
    out = np.zeros((B, S, D), dtype=np.float32)
    for c in range(NCORES):
        out[c // HGROUPS] += res.results[c]["out"]
    out += b_o
    return out
